# revision 3
# baseline (speedup 1.0000x reference)
"""Trainium2 Bass kernel v3.2 for nn_LongTextEncoder (attention-pool + segment mean).

Same math as the baseline (all fp32), restructured for PE efficiency:
  - Host permutes each chunk's L rows unmasked-first; scores/pooled only
    process ceil(K/128) of the 4 l-tiles (mask folds in as an additive
    -1e4 on scores before exp, so padding tiles give exact zero alphas).
  - Chunks are dealt to cores by sorted K so one SPMD program (baked
    per-slot tile counts = max over cores) fits all 8 cores.
  - Phase A (per-chunk sums) and the pooled matmuls accumulate into
    PSUM rows 32*i+sub via 2-col one-hot / alpha lhsT at 4 column tile
    positions; matmul issue is interleaved across 4 chunks so the PE
    runs 4 concurrent column-group streams (measured 3.6x).
  - Phase B: m gathered to [8,768] via small SWDGE DMAs, PE transposes
    to columns, r = AT.T @ m as matmuls with the tiny mcols stationary;
    rA/rB run in different column groups; +c rides as a 9th selector
    row via a small DMA.
  - den = sum(alpha) via exp's accum_out + a one-hot [128,2] matmul
    into a spare PSUM column.
"""

import math
import os
import sys

import numpy as np

for _p in (
    "/root/.axon_site",
    "/root/.axon_site/_ro/trn_rl_repo",
    "/root/.axon_site/_ro/pypackages",
    "/opt/trn_rl_repo",
    "/opt/pypackages",
):
    if os.path.isdir(_p) and _p not in sys.path:
        sys.path.append(_p)

import concourse.bass as bass
import concourse.tile as tile
from concourse import bacc
from concourse import mybir
from concourse.bass_utils import run_bass_kernel_spmd
from concourse.masks import make_identity

NCORES = 8
N, L, H = 512, 512, 768
NS = N // NCORES
G = 8
NG = NS // G
HC = H // 128
LT = L // 128
F32 = mybir.dt.float32
ACT = mybir.ActivationFunctionType
OP = mybir.AluOpType

_CACHE: dict = {}


def _build_bass(kts, diag=False):
    """kts: tuple of NS ints (tiles per chunk slot, 1..4)."""
    nc = bacc.Bacc(trn_type="TRN2")
    hs_d = nc.declare_dram_parameter("hs", [NS, L, H], F32, isOutput=False)
    mkb_d = nc.declare_dram_parameter("mkb", [128, LT, NS], F32, isOutput=False)
    sel_d = nc.declare_dram_parameter("sel9", [9, G, 128], F32, isOutput=False)
    oh_d = nc.declare_dram_parameter("oh", [128, 4], F32, isOutput=False)
    at_d = nc.declare_dram_parameter("AT", [H, H], F32, isOutput=False)
    cr_d = nc.declare_dram_parameter("crow", [1, H], F32, isOutput=False)
    out_d = nc.declare_dram_parameter("out", [NS, H], F32, isOutput=True)

    with tile.TileContext(nc) as tc:
        with (
            tc.tile_pool(name="consts", bufs=1) as consts,
            tc.tile_pool(name="hspool", bufs=11) as hspool,
            tc.tile_pool(name="sm", bufs=2) as sm,
            tc.tile_pool(name="rbs", bufs=3) as rbs,
            tc.tile_pool(name="jk", bufs=2) as jk,
            tc.tile_pool(name="msc", bufs=6) as msc,
            tc.tile_pool(name="psM", bufs=1, space="PSUM") as psM,
            tc.tile_pool(name="psP", bufs=1, space="PSUM") as psP,
            tc.tile_pool(name="psRb", bufs=1, space="PSUM") as psRb,
            tc.tile_pool(name="psB", bufs=1, space="PSUM") as psB,
        ):
            # ---- constants ----
            at_t = consts.tile([128, HC, H], F32)
            nc.sync.dma_start(out=at_t, in_=at_d.rearrange("(a p) h -> p a h", p=128))
            mkb_t = consts.tile([128, LT, NS], F32)
            nc.sync.dma_start(out=mkb_t, in_=mkb_d[:, :, :])
            sel9 = consts.tile([9, G, 128], F32)
            nc.sync.dma_start(out=sel9, in_=sel_d[:, :, :])
            oh_t = consts.tile([128, 4], F32)
            nc.sync.dma_start(out=oh_t, in_=oh_d[:, :])
            ident = consts.tile([128, 128], F32)
            make_identity(nc, ident)
            ones_col = consts.tile([128, 1], F32)
            nc.gpsimd.memset(ones_col, 1.0)
            zeros128 = consts.tile([128, 128], F32)
            nc.gpsimd.memset(zeros128, 0.0)

            def zero_bank(ps):
                # full-width zeroing matmul: sole start=True of the bank's
                # bracket; serializes ahead of the packed accumulators.
                nc.tensor.matmul(
                    out=ps[:, 0:512],
                    lhsT=zeros128,
                    rhs=at_t[:, 0, 0:512],
                    start=True,
                    stop=False,
                )

            hs_tiles: dict = {}

            def load_chunk(n):
                t = hspool.tile([128, LT, H], F32, tag="hs", name=f"hs{n}")
                eng = nc.sync if n % 2 == 0 else nc.scalar
                eng.dma_start(
                    out=t, in_=hs_d[n].rearrange("(t p) h -> p t h", p=128)
                )
                hs_tiles[n] = t
                return t

            def phase_a_batch(cs, mA, mB):
                # issue interleaved across chunks -> 4 concurrent col groups
                # ONE accumulation bracket per PSUM bank: start only on the
                # very first MM (start clears has_written bank-wide).
                for lt in range(LT):
                    for bank in range(2):
                        for c in cs:
                            i, sub = c % 4, c // 4
                            lhsT = oh_t[:, 0:2] if sub == 0 else oh_t[:, 2:4]
                            r0 = 32 * i
                            st = False
                            sp = c == 7 and lt == LT - 1
                            t = hs_tiles[phase_a_batch.base + c]
                            if bank == 0:
                                nc.tensor.matmul(
                                    out=mA[r0 : r0 + 2, :],
                                    lhsT=lhsT,
                                    rhs=t[:, lt, 0:512],
                                    start=st,
                                    stop=sp,
                                    tile_position=(0, r0),
                                )
                            else:
                                nc.tensor.matmul(
                                    out=mB[r0 : r0 + 2, 0:256],
                                    lhsT=lhsT,
                                    rhs=t[:, lt, 512:768],
                                    start=st,
                                    stop=sp,
                                    tile_position=(0, r0),
                                )

            def new_m_tiles(g):
                mA = psM.tile([128, 512], F32, tag="mA", name=f"mA{g}")
                mB = psM.tile([128, 512], F32, tag="mB", name=f"mB{g}")
                zero_bank(mA)
                zero_bank(mB)
                return mA, mB

            # ---- bootstrap: group 0 loads + phase A ----
            mcur = new_m_tiles(0)
            for c in range(G):
                load_chunk(c)
            phase_a_batch.base = 0
            phase_a_batch(range(0, 4), *mcur)
            phase_a_batch(range(4, 8), *mcur)

            ngroups = 1 if diag else NG
            for g in range(ngroups):
                mA, mB = mcur
                # ---- phase B for group g ----
                m_sprd = sm.tile([128, H], F32, tag="msprd")
                for i in range(4):
                    r0 = 32 * i
                    nc.scalar.activation(
                        out=m_sprd[r0 : r0 + 2, 0:512],
                        in_=mA[r0 : r0 + 2, :],
                        func=ACT.Copy,
                    )
                    nc.scalar.activation(
                        out=m_sprd[r0 : r0 + 2, 512:768],
                        in_=mB[r0 : r0 + 2, 0:256],
                        func=ACT.Copy,
                    )
                m_sb = sm.tile([G, H], F32, tag="msb")
                for i in range(4):
                    nc.gpsimd.dma_start(
                        out=m_sb[2 * i : 2 * i + 2, :],
                        in_=m_sprd[32 * i : 32 * i + 2, :],
                    )
                if g + 1 < ngroups:
                    mcur = new_m_tiles(g + 1)

                mc_ps = psB.tile([128, HC, G], F32, tag="pB1")
                for hb in range(HC):
                    nc.tensor.transpose(
                        out=mc_ps[:, hb, :],
                        in_=m_sb[:, hb * 128 : (hb + 1) * 128],
                        identity=ident[:G, :G],
                    )
                mcols = sm.tile([128, HC, G], F32, tag="mcols")
                nc.scalar.activation(out=mcols, in_=mc_ps, func=ACT.Copy)

                r_rows9 = sm.tile([9, H], F32, tag="r9")
                rA = psB.tile([G, 512], F32, tag="pB2", name=f"rA{g}")
                rBt = psRb.tile([40, 512], F32, tag="rbB", name=f"rBt{g}")
                for hb in range(HC):
                    nc.tensor.matmul(
                        out=rA,
                        lhsT=mcols[:, hb, :],
                        rhs=at_t[:, hb, 0:512],
                        start=(hb == 0),
                        stop=(hb == HC - 1),
                    )
                    nc.tensor.matmul(
                        out=rBt[32:40, 0:256],
                        lhsT=mcols[:, hb, :],
                        rhs=at_t[:, hb, 512:768],
                        start=(hb == 0),
                        stop=(hb == HC - 1),
                        tile_position=(0, 32),
                    )
                nc.scalar.activation(out=r_rows9[0:8, 0:512], in_=rA, func=ACT.Copy)
                rb_sprd = sm.tile([40, 256], F32, tag="rbsprd")
                nc.scalar.activation(
                    out=rb_sprd[32:40, :], in_=rBt[32:40, 0:256], func=ACT.Copy
                )
                nc.gpsimd.dma_start(
                    out=r_rows9[0:8, 512:768], in_=rb_sprd[32:40, :]
                )
                nc.gpsimd.dma_start(out=r_rows9[8:9, :], in_=cr_d[:, :])

                if diag and g == 0:
                    nc.gpsimd.dma_start(out=out_d[8:16, :], in_=m_sb)
                    nc.gpsimd.dma_start(out=out_d[16:25, :], in_=r_rows9)
                plA = psP.tile([128, 512], F32, tag="plA", name=f"plA{g}")
                plB = psP.tile([128, 512], F32, tag="plB", name=f"plB{g}")
                zero_bank(plA)
                zero_bank(plB)
                den_sp = sm.tile([128, 1], F32, tag="densp")
                nc.gpsimd.memset(den_sp, 1.0)

                # ---- chunk loop ----
                mescs = {}
                pdens = {}

                def pooled_batch(cs):
                    # (lt, c) issue list; single bracket per bank across both
                    # half-group batches: start on global first, stop on the
                    # globally last pooled MM (plA) / last den MM (plB).
                    kt_of = lambda c: kts[g * G + c]
                    all_lc = [
                        (l2, c2)
                        for l2 in range(LT)
                        for c2 in range(4, 8)
                        if l2 < kt_of(c2)
                    ]
                    for lt in range(LT):
                        for bank in range(2):
                            for c in cs:
                                n = g * G + c
                                kt = kt_of(c)
                                if lt >= kt:
                                    continue
                                i, sub = c % 4, c // 4
                                r0 = 32 * i
                                st = False
                                sp = cs[0] == 4 and (lt, c) == all_lc[-1]
                                t = hs_tiles[n]
                                if bank == 0:
                                    nc.tensor.matmul(
                                        out=plA[r0 : r0 + 2, :],
                                        lhsT=mescs[c][:, lt, :],
                                        rhs=t[:, lt, 0:512],
                                        start=st,
                                        stop=sp,
                                        tile_position=(0, r0),
                                    )
                                else:
                                    nc.tensor.matmul(
                                        out=plB[r0 : r0 + 2, 0:256],
                                        lhsT=mescs[c][:, lt, :],
                                        rhs=t[:, lt, 512:768],
                                        start=st,
                                        stop=sp,
                                        tile_position=(0, r0),
                                    )
                    for c in cs:
                        i, sub = c % 4, c // 4
                        r0 = 32 * i
                        nc.tensor.matmul(
                            out=plB[r0 : r0 + 2, 300:301],
                            lhsT=pdens[c],
                            rhs=ones_col,
                            start=False,
                            stop=(c == 7),
                            tile_position=(0, r0),
                        )
                    # release hs tiles of this batch
                    for c in cs:
                        hs_tiles.pop(g * G + c)

                for c in range(G):
                    n = g * G + c
                    kt = kts[n]
                    i, sub = c % 4, c // 4
                    t = hs_tiles[n]

                    rbA = psRb.tile([128, 512], F32, tag="rbA", name=f"rbA{n}")
                    rbB = psRb.tile([128, 512], F32, tag="rbB", name=f"rbB{n}")
                    nc.tensor.matmul(
                        out=rbA,
                        lhsT=sel9[:, c, :],
                        rhs=r_rows9[:, 0:512],
                        start=True,
                        stop=True,
                    )
                    nc.tensor.matmul(
                        out=rbB[:, 0:256],
                        lhsT=sel9[:, c, :],
                        rhs=r_rows9[:, 512:768],
                        start=True,
                        stop=True,
                    )
                    rb_s = rbs.tile([128, H], F32, tag="rb", name=f"rb{n}")
                    nc.scalar.activation(out=rb_s[:, 0:512], in_=rbA, func=ACT.Copy)
                    nc.scalar.activation(
                        out=rb_s[:, 512:768], in_=rbB[:, 0:256], func=ACT.Copy
                    )

                    sc_t = sm.tile([128, LT], F32, tag="sc", name=f"sc{n}")
                    for lt in range(kt):
                        junk = jk.tile([128, H], F32, tag="junk")
                        nc.vector.scalar_tensor_tensor(
                            out=junk,
                            in0=t[:, lt, :],
                            scalar=1.0,
                            in1=rb_s,
                            op0=OP.mult,
                            op1=OP.mult,
                            accum_out=sc_t[:, lt : lt + 1],
                        )
                    sc_m = sm.tile([128, LT], F32, tag="scm", name=f"scm{n}")
                    nc.vector.tensor_add(
                        sc_m[:, 0:kt], sc_t[:, 0:kt], mkb_t[:, 0:kt, n % NS]
                    )

                    if diag and g == 0 and c == 0:
                        nc.gpsimd.dma_start(out=out_d[25:26, :], in_=rb_s[0:1, :])
                        nc.gpsimd.dma_start(out=out_d[26:27, :], in_=rb_s[127:128, :])
                        nc.gpsimd.dma_start(
                            out=out_d[27:28, 0:512].rearrange("x (p f) -> (x p) f", p=128),
                            in_=sc_t[:, 0:4],
                        )
                    mesc = msc.tile([128, LT, 2], F32, tag="mesc", name=f"me{n}")
                    nc.gpsimd.memset(mesc, 0.0)
                    pden = msc.tile([128, 2], F32, tag="pden", name=f"pd{n}")
                    nc.gpsimd.memset(pden, 0.0)
                    nc.scalar.activation(
                        out=mesc[:, 0:kt, sub],
                        in_=sc_m[:, 0:kt],
                        func=ACT.Exp,
                        accum_out=pden[:, sub : sub + 1],
                    )
                    if diag and g == 0 and c == 0:
                        nc.gpsimd.dma_start(
                            out=out_d[28:29, 0:512].rearrange("x (p f) -> (x p) f", p=128),
                            in_=mesc[:, :, 0],
                        )
                    mescs[c] = mesc
                    pdens[c] = pden

                    # batched pooled issue at half-group boundaries
                    if c == 3:
                        pooled_batch(range(0, 4))
                    elif c == 7:
                        pooled_batch(range(4, 8))

                    # prefetch next group's chunk + batched phase A
                    if g + 1 < ngroups:
                        load_chunk((g + 1) * G + c)
                        if c == 3:
                            phase_a_batch.base = (g + 1) * G
                            phase_a_batch(range(0, 4), *mcur)
                        elif c == 7:
                            phase_a_batch.base = (g + 1) * G
                            phase_a_batch(range(4, 8), *mcur)

                # ---- output for group g ----
                for i in range(4):
                    r0 = 32 * i
                    nc.scalar.activation(
                        out=den_sp[r0 : r0 + 2, :],
                        in_=plB[r0 : r0 + 2, 300:301],
                        func=ACT.Copy,
                    )
                rden = sm.tile([128, 1], F32, tag="rden")
                nc.vector.reciprocal(out=rden, in_=den_sp)
                po_sp = sm.tile([128, H], F32, tag="posp")
                for i in range(4):
                    r0 = 32 * i
                    nc.scalar.activation(
                        out=po_sp[r0 : r0 + 2, 0:512],
                        in_=plA[r0 : r0 + 2, :],
                        func=ACT.Copy,
                        scale=rden[r0 : r0 + 2, :],
                    )
                    nc.scalar.activation(
                        out=po_sp[r0 : r0 + 2, 512:768],
                        in_=plB[r0 : r0 + 2, 0:256],
                        func=ACT.Copy,
                        scale=rden[r0 : r0 + 2, :],
                    )
                for i in range(4):
                    nc.gpsimd.dma_start(
                        out=out_d[g * G : (g + 1) * G].rearrange(
                            "(s i) h -> i s h", s=2
                        )[i],
                        in_=po_sp[32 * i : 32 * i + 2, :],
                    )

    if not nc.is_finalized():
        nc.finalize()
    return nc


def _get_nc(kts):
    key = ("nc3", kts)
    if key not in _CACHE:
        _CACHE[key] = _build_bass(kts)
    return _CACHE[key]


def _prepare(hidden_states, attention_mask, Wq, bq, Wk, bk):
    hs = np.asarray(hidden_states, dtype=np.float32)
    mask = np.asarray(attention_mask).astype(np.int32)
    Wq = np.asarray(Wq, dtype=np.float32)
    bq = np.asarray(bq, dtype=np.float32)
    Wk = np.asarray(Wk, dtype=np.float32)

    AT = ((Wq.T @ Wk) / np.float32(L * math.sqrt(H))).astype(np.float32)
    crow = ((Wk.T @ bq) / np.float32(math.sqrt(H))).astype(np.float32).reshape(1, H)

    K = mask.sum(axis=1)  # unmasked count per chunk [N]
    # deal chunks to cores by sorted K: slot j of every core has similar K
    order = np.argsort(K, kind="stable")  # ascending
    assign = order.reshape(NS, NCORES)  # slot j -> chunks assign[j]
    kts = []
    for j in range(NS):
        kmax = int(K[assign[j]].max())
        kts.append(min(LT, max(1, -(-kmax // 128))))
    kts = tuple(kts)

    in_maps = []
    chunk_of = np.zeros((NCORES, NS), np.int64)
    for core in range(NCORES):
        chunks = assign[:, core]
        chunk_of[core] = chunks
        hs_c = np.empty((NS, L, H), np.float32)
        mkb = np.zeros((128, LT, NS), np.float32)
        for j, n in enumerate(chunks):
            perm = np.argsort(1 - mask[n], kind="stable")
            hs_c[j] = hs[n][perm]
            mperm = mask[n][perm]
            mkb[:, :, j] = np.where(
                mperm.reshape(LT, 128).T == 0, np.float32(-1e4), np.float32(0.0)
            )
        sel9 = np.zeros((9, G, 128), np.float32)
        for c in range(G):
            sel9[2 * (c % 4) + c // 4, c, :] = 1.0
        sel9[8, :, :] = 1.0
        oh = np.zeros((128, 4), np.float32)
        oh[:, 0] = 1.0
        oh[:, 3] = 1.0
        in_maps.append(
            {
                "hs": hs_c,
                "mkb": mkb,
                "sel9": sel9,
                "oh": oh,
                "AT": AT,
                "crow": crow,
            }
        )
    return in_maps, chunk_of, kts


def run_on_device(hidden_states, attention_mask, Wq, bq, Wk, bk, trace=False):
    in_maps, chunk_of, kts = _prepare(
        hidden_states, attention_mask, Wq, bq, Wk, bk
    )
    nc = _get_nc(kts)
    res = run_bass_kernel_spmd(nc, in_maps, core_ids=list(range(NCORES)), trace=trace)
    pooled = np.zeros((N, H), np.float32)
    for core in range(NCORES):
        pooled[chunk_of[core]] = res.results[core]["out"]
    return pooled, res


def kernel(hidden_states, attention_mask, sample_map, Wq, bq, Wk, bk, num_texts):
    pooled, _ = run_on_device(hidden_states, attention_mask, Wq, bq, Wk, bk)
    smap = np.asarray(sample_map).astype(np.int64)
    T = int(num_texts)
    sums = np.zeros((T, H), np.float32)
    np.add.at(sums, smap, pooled)
    counts = np.bincount(smap, minlength=T).astype(np.float32)
    counts = np.clip(counts, 1.0, None)
    return (sums / counts[:, None]).astype(np.float32)


if __name__ == "__main__":
    nc = _build_bass(tuple([3] * NS))
    print("built ok")


# revision 4
# speedup vs baseline: 1.2250x; 1.2250x over previous
"""Trainium2 Bass kernel v3.2 for nn_LongTextEncoder (attention-pool + segment mean).

Same math as the baseline (all fp32), restructured for PE efficiency:
  - Host permutes each chunk's L rows unmasked-first; scores/pooled only
    process ceil(K/128) of the 4 l-tiles (mask folds in as an additive
    -1e4 on scores before exp, so padding tiles give exact zero alphas).
  - Chunks are dealt to cores by sorted K so one SPMD program (baked
    per-slot tile counts = max over cores) fits all 8 cores.
  - Phase A (per-chunk sums) and the pooled matmuls accumulate into
    PSUM rows 32*i+sub via 2-col one-hot / alpha lhsT at 4 column tile
    positions; matmul issue is interleaved across 4 chunks so the PE
    runs 4 concurrent column-group streams (measured 3.6x).
  - Phase B: m gathered to [8,768] via small SWDGE DMAs, PE transposes
    to columns, r = AT.T @ m as matmuls with the tiny mcols stationary;
    rA/rB run in different column groups; +c rides as a 9th selector
    row via a small DMA.
  - den = sum(alpha) via exp's accum_out + a one-hot [128,2] matmul
    into a spare PSUM column.
"""

import math
import os
import sys

import numpy as np

for _p in (
    "/root/.axon_site",
    "/root/.axon_site/_ro/trn_rl_repo",
    "/root/.axon_site/_ro/pypackages",
    "/opt/trn_rl_repo",
    "/opt/pypackages",
):
    if os.path.isdir(_p) and _p not in sys.path:
        sys.path.append(_p)

import concourse.bass as bass
import concourse.tile as tile
from concourse import bacc
from concourse import mybir
from concourse.bass_utils import run_bass_kernel_spmd
from concourse.masks import make_identity

NCORES = 8
N, L, H = 512, 512, 768
NS = N // NCORES
G = 8
NG = NS // G
HC = H // 128
LT = L // 128
F32 = mybir.dt.float32
ACT = mybir.ActivationFunctionType
OP = mybir.AluOpType

_CACHE: dict = {}


def _build_bass(kts, diag=False):
    """kts: tuple of NS ints (tiles per chunk slot, 1..4)."""
    nc = bacc.Bacc(trn_type="TRN2")
    hs_d = nc.declare_dram_parameter("hs", [NS, L, H], F32, isOutput=False)
    mkb_d = nc.declare_dram_parameter("mkb", [128, LT, NS], F32, isOutput=False)
    sel_d = nc.declare_dram_parameter("sel9", [9, G, 128], F32, isOutput=False)
    oh_d = nc.declare_dram_parameter("oh", [128, 4], F32, isOutput=False)
    at_d = nc.declare_dram_parameter("AT", [H, H], F32, isOutput=False)
    cr_d = nc.declare_dram_parameter("crow", [1, H], F32, isOutput=False)
    out_d = nc.declare_dram_parameter("out", [NS, H], F32, isOutput=True)

    with tile.TileContext(nc) as tc:
        with (
            tc.tile_pool(name="consts", bufs=1) as consts,
            tc.tile_pool(name="hspool", bufs=11) as hspool,
            tc.tile_pool(name="sm", bufs=2) as sm,
            tc.tile_pool(name="rbs", bufs=3) as rbs,
            tc.tile_pool(name="jk", bufs=2) as jk,
            tc.tile_pool(name="msc", bufs=6) as msc,
            tc.tile_pool(name="psM", bufs=1, space="PSUM") as psM,
            tc.tile_pool(name="psP", bufs=1, space="PSUM") as psP,
            tc.tile_pool(name="psRb", bufs=1, space="PSUM") as psRb,
            tc.tile_pool(name="psB", bufs=1, space="PSUM") as psB,
        ):
            # ---- constants ----
            at_t = consts.tile([128, HC, H], F32)
            nc.sync.dma_start(out=at_t, in_=at_d.rearrange("(a p) h -> p a h", p=128))
            mkb_t = consts.tile([128, LT, NS], F32)
            nc.sync.dma_start(out=mkb_t, in_=mkb_d[:, :, :])
            sel9 = consts.tile([9, G, 128], F32)
            nc.sync.dma_start(out=sel9, in_=sel_d[:, :, :])
            oh_t = consts.tile([128, 4], F32)
            nc.sync.dma_start(out=oh_t, in_=oh_d[:, :])
            ident = consts.tile([128, 128], F32)
            make_identity(nc, ident)
            ones_col = consts.tile([128, 1], F32)
            nc.gpsimd.memset(ones_col, 1.0)
            zeros128 = consts.tile([128, 128], F32)
            nc.gpsimd.memset(zeros128, 0.0)

            def zero_bank(ps):
                # full-width zeroing matmul: sole start=True of the bank's
                # bracket; serializes ahead of the packed accumulators.
                nc.tensor.matmul(
                    out=ps[:, 0:512],
                    lhsT=zeros128,
                    rhs=at_t[:, 0, 0:512],
                    start=True,
                    stop=False,
                )

            hs_tiles: dict = {}

            def load_chunk(n):
                t = hspool.tile([128, LT, H], F32, tag="hs", name=f"hs{n}")
                nc.sync.dma_start(
                    out=t, in_=hs_d[n].rearrange("(t p) h -> p t h", p=128)
                )
                hs_tiles[n] = t
                return t

            def phase_a_batch(cs, mA, mB):
                # issue interleaved across chunks -> 4 concurrent col groups
                # ONE accumulation bracket per PSUM bank: start only on the
                # very first MM (start clears has_written bank-wide).
                for lt in range(LT):
                    for bank in range(2):
                        for c in cs:
                            i, sub = c % 4, c // 4
                            lhsT = oh_t[:, 0:2] if sub == 0 else oh_t[:, 2:4]
                            r0 = 32 * i
                            st = False
                            sp = c == 7 and lt == LT - 1
                            t = hs_tiles[phase_a_batch.base + c]
                            if bank == 0:
                                nc.tensor.matmul(
                                    out=mA[r0 : r0 + 2, :],
                                    lhsT=lhsT,
                                    rhs=t[:, lt, 0:512],
                                    start=st,
                                    stop=sp,
                                    tile_position=(0, r0),
                                )
                            else:
                                nc.tensor.matmul(
                                    out=mB[r0 : r0 + 2, 0:256],
                                    lhsT=lhsT,
                                    rhs=t[:, lt, 512:768],
                                    start=st,
                                    stop=sp,
                                    tile_position=(0, r0),
                                )

            def new_m_tiles(g):
                mA = psM.tile([128, 512], F32, tag="mA", name=f"mA{g}")
                mB = psM.tile([128, 512], F32, tag="mB", name=f"mB{g}")
                zero_bank(mA)
                zero_bank(mB)
                return mA, mB

            # ---- bootstrap: group 0 loads + phase A ----
            mcur = new_m_tiles(0)
            for c in range(G):
                load_chunk(c)
            phase_a_batch.base = 0
            phase_a_batch(range(0, 4), *mcur)
            phase_a_batch(range(4, 8), *mcur)

            ngroups = 1 if diag else NG
            for g in range(ngroups):
                mA, mB = mcur
                # ---- phase B for group g ----
                m_sprd = sm.tile([128, H], F32, tag="msprd")
                for i in range(4):
                    r0 = 32 * i
                    nc.scalar.activation(
                        out=m_sprd[r0 : r0 + 2, 0:512],
                        in_=mA[r0 : r0 + 2, :],
                        func=ACT.Copy,
                    )
                    nc.scalar.activation(
                        out=m_sprd[r0 : r0 + 2, 512:768],
                        in_=mB[r0 : r0 + 2, 0:256],
                        func=ACT.Copy,
                    )
                m_sb = sm.tile([G, H], F32, tag="msb")
                for i in range(4):
                    nc.gpsimd.dma_start(
                        out=m_sb[2 * i : 2 * i + 2, :],
                        in_=m_sprd[32 * i : 32 * i + 2, :],
                    )
                if g + 1 < ngroups:
                    mcur = new_m_tiles(g + 1)

                mc_ps = psB.tile([128, HC, G], F32, tag="pB1")
                for hb in range(HC):
                    nc.tensor.transpose(
                        out=mc_ps[:, hb, :],
                        in_=m_sb[:, hb * 128 : (hb + 1) * 128],
                        identity=ident[:G, :G],
                    )
                mcols = sm.tile([128, HC, G], F32, tag="mcols")
                nc.scalar.activation(out=mcols, in_=mc_ps, func=ACT.Copy)

                r_rows9 = sm.tile([9, H], F32, tag="r9")
                rA = psB.tile([G, 512], F32, tag="pB2", name=f"rA{g}")
                rBt = psRb.tile([40, 512], F32, tag="rbB", name=f"rBt{g}")
                for hb in range(HC):
                    nc.tensor.matmul(
                        out=rA,
                        lhsT=mcols[:, hb, :],
                        rhs=at_t[:, hb, 0:512],
                        start=(hb == 0),
                        stop=(hb == HC - 1),
                    )
                    nc.tensor.matmul(
                        out=rBt[32:40, 0:256],
                        lhsT=mcols[:, hb, :],
                        rhs=at_t[:, hb, 512:768],
                        start=(hb == 0),
                        stop=(hb == HC - 1),
                        tile_position=(0, 32),
                    )
                nc.scalar.activation(out=r_rows9[0:8, 0:512], in_=rA, func=ACT.Copy)
                rb_sprd = sm.tile([40, 256], F32, tag="rbsprd")
                nc.scalar.activation(
                    out=rb_sprd[32:40, :], in_=rBt[32:40, 0:256], func=ACT.Copy
                )
                nc.gpsimd.dma_start(
                    out=r_rows9[0:8, 512:768], in_=rb_sprd[32:40, :]
                )
                nc.gpsimd.dma_start(out=r_rows9[8:9, :], in_=cr_d[:, :])

                if diag and g == 0:
                    nc.gpsimd.dma_start(out=out_d[8:16, :], in_=m_sb)
                    nc.gpsimd.dma_start(out=out_d[16:25, :], in_=r_rows9)
                plA = psP.tile([128, 512], F32, tag="plA", name=f"plA{g}")
                plB = psP.tile([128, 512], F32, tag="plB", name=f"plB{g}")
                zero_bank(plA)
                zero_bank(plB)
                den_sp = sm.tile([128, 1], F32, tag="densp")
                nc.gpsimd.memset(den_sp, 1.0)

                # ---- chunk loop ----
                mescs = {}
                pdens = {}

                def pooled_batch(cs):
                    # (lt, c) issue list; single bracket per bank across both
                    # half-group batches: start on global first, stop on the
                    # globally last pooled MM (plA) / last den MM (plB).
                    kt_of = lambda c: kts[g * G + c]
                    all_lc = [
                        (l2, c2)
                        for l2 in range(LT)
                        for c2 in range(4, 8)
                        if l2 < kt_of(c2)
                    ]
                    for lt in range(LT):
                        for bank in range(2):
                            for c in cs:
                                n = g * G + c
                                kt = kt_of(c)
                                if lt >= kt:
                                    continue
                                i, sub = c % 4, c // 4
                                r0 = 32 * i
                                st = False
                                sp = cs[0] == 4 and (lt, c) == all_lc[-1]
                                t = hs_tiles[n]
                                if bank == 0:
                                    nc.tensor.matmul(
                                        out=plA[r0 : r0 + 2, :],
                                        lhsT=mescs[c][:, lt, :],
                                        rhs=t[:, lt, 0:512],
                                        start=st,
                                        stop=sp,
                                        tile_position=(0, r0),
                                    )
                                else:
                                    nc.tensor.matmul(
                                        out=plB[r0 : r0 + 2, 0:256],
                                        lhsT=mescs[c][:, lt, :],
                                        rhs=t[:, lt, 512:768],
                                        start=st,
                                        stop=sp,
                                        tile_position=(0, r0),
                                    )
                    for c in cs:
                        i, sub = c % 4, c // 4
                        r0 = 32 * i
                        nc.tensor.matmul(
                            out=plB[r0 : r0 + 2, 300:301],
                            lhsT=pdens[c],
                            rhs=ones_col,
                            start=False,
                            stop=(c == 7),
                            tile_position=(0, r0),
                        )
                    # release hs tiles of this batch
                    for c in cs:
                        hs_tiles.pop(g * G + c)

                for c in range(G):
                    n = g * G + c
                    kt = kts[n]
                    i, sub = c % 4, c // 4
                    t = hs_tiles[n]

                    rbA = psRb.tile([128, 512], F32, tag="rbA", name=f"rbA{n}")
                    rbB = psRb.tile([128, 512], F32, tag="rbB", name=f"rbB{n}")
                    nc.tensor.matmul(
                        out=rbA,
                        lhsT=sel9[:, c, :],
                        rhs=r_rows9[:, 0:512],
                        start=True,
                        stop=True,
                    )
                    nc.tensor.matmul(
                        out=rbB[:, 0:256],
                        lhsT=sel9[:, c, :],
                        rhs=r_rows9[:, 512:768],
                        start=True,
                        stop=True,
                    )
                    rb_s = rbs.tile([128, H], F32, tag="rb", name=f"rb{n}")
                    nc.scalar.activation(out=rb_s[:, 0:512], in_=rbA, func=ACT.Copy)
                    nc.scalar.activation(
                        out=rb_s[:, 512:768], in_=rbB[:, 0:256], func=ACT.Copy
                    )

                    sc_t = sm.tile([128, LT], F32, tag="sc", name=f"sc{n}")
                    for lt in range(kt):
                        junk = jk.tile([128, H], F32, tag="junk")
                        nc.vector.scalar_tensor_tensor(
                            out=junk,
                            in0=t[:, lt, :],
                            scalar=1.0,
                            in1=rb_s,
                            op0=OP.mult,
                            op1=OP.mult,
                            accum_out=sc_t[:, lt : lt + 1],
                        )
                    sc_m = sm.tile([128, LT], F32, tag="scm", name=f"scm{n}")
                    nc.vector.tensor_add(
                        sc_m[:, 0:kt], sc_t[:, 0:kt], mkb_t[:, 0:kt, n % NS]
                    )

                    if diag and g == 0 and c == 0:
                        nc.gpsimd.dma_start(out=out_d[25:26, :], in_=rb_s[0:1, :])
                        nc.gpsimd.dma_start(out=out_d[26:27, :], in_=rb_s[127:128, :])
                        nc.gpsimd.dma_start(
                            out=out_d[27:28, 0:512].rearrange("x (p f) -> (x p) f", p=128),
                            in_=sc_t[:, 0:4],
                        )
                    mesc = msc.tile([128, LT, 2], F32, tag="mesc", name=f"me{n}")
                    nc.gpsimd.memset(mesc, 0.0)
                    pden = msc.tile([128, 2], F32, tag="pden", name=f"pd{n}")
                    nc.gpsimd.memset(pden, 0.0)
                    nc.scalar.activation(
                        out=mesc[:, 0:kt, sub],
                        in_=sc_m[:, 0:kt],
                        func=ACT.Exp,
                        accum_out=pden[:, sub : sub + 1],
                    )
                    if diag and g == 0 and c == 0:
                        nc.gpsimd.dma_start(
                            out=out_d[28:29, 0:512].rearrange("x (p f) -> (x p) f", p=128),
                            in_=mesc[:, :, 0],
                        )
                    mescs[c] = mesc
                    pdens[c] = pden

                    # batched pooled issue at half-group boundaries
                    if c == 3:
                        pooled_batch(range(0, 4))
                    elif c == 7:
                        pooled_batch(range(4, 8))

                    # prefetch next group's chunk + batched phase A
                    if g + 1 < ngroups:
                        load_chunk((g + 1) * G + c)
                        if c == 3:
                            phase_a_batch.base = (g + 1) * G
                            phase_a_batch(range(0, 4), *mcur)
                        elif c == 7:
                            phase_a_batch.base = (g + 1) * G
                            phase_a_batch(range(4, 8), *mcur)

                # ---- output for group g ----
                for i in range(4):
                    r0 = 32 * i
                    nc.scalar.activation(
                        out=den_sp[r0 : r0 + 2, :],
                        in_=plB[r0 : r0 + 2, 300:301],
                        func=ACT.Copy,
                    )
                rden = sm.tile([128, 1], F32, tag="rden")
                nc.vector.reciprocal(out=rden, in_=den_sp)
                po_sp = sm.tile([128, H], F32, tag="posp")
                for i in range(4):
                    r0 = 32 * i
                    nc.scalar.activation(
                        out=po_sp[r0 : r0 + 2, 0:512],
                        in_=plA[r0 : r0 + 2, :],
                        func=ACT.Copy,
                        scale=rden[r0 : r0 + 2, :],
                    )
                    nc.scalar.activation(
                        out=po_sp[r0 : r0 + 2, 512:768],
                        in_=plB[r0 : r0 + 2, 0:256],
                        func=ACT.Copy,
                        scale=rden[r0 : r0 + 2, :],
                    )
                for i in range(4):
                    nc.gpsimd.dma_start(
                        out=out_d[g * G : (g + 1) * G].rearrange(
                            "(s i) h -> i s h", s=2
                        )[i],
                        in_=po_sp[32 * i : 32 * i + 2, :],
                    )

    if not nc.is_finalized():
        nc.finalize()
    return nc


def _get_nc(kts):
    key = ("nc3", kts)
    if key not in _CACHE:
        _CACHE[key] = _build_bass(kts)
    return _CACHE[key]


def _prepare(hidden_states, attention_mask, Wq, bq, Wk, bk):
    hs = np.asarray(hidden_states, dtype=np.float32)
    mask = np.asarray(attention_mask).astype(np.int32)
    Wq = np.asarray(Wq, dtype=np.float32)
    bq = np.asarray(bq, dtype=np.float32)
    Wk = np.asarray(Wk, dtype=np.float32)

    AT = ((Wq.T @ Wk) / np.float32(L * math.sqrt(H))).astype(np.float32)
    crow = ((Wk.T @ bq) / np.float32(math.sqrt(H))).astype(np.float32).reshape(1, H)

    K = mask.sum(axis=1)  # unmasked count per chunk [N]
    # deal chunks to cores by sorted K: slot j of every core has similar K
    order = np.argsort(K, kind="stable")  # ascending
    assign = order.reshape(NS, NCORES)  # slot j -> chunks assign[j]
    kts = []
    for j in range(NS):
        kmax = int(K[assign[j]].max())
        kts.append(min(LT, max(1, -(-kmax // 128))))
    kts = tuple(kts)

    in_maps = []
    chunk_of = np.zeros((NCORES, NS), np.int64)
    for core in range(NCORES):
        chunks = assign[:, core]
        chunk_of[core] = chunks
        hs_c = np.empty((NS, L, H), np.float32)
        mkb = np.zeros((128, LT, NS), np.float32)
        for j, n in enumerate(chunks):
            perm = np.argsort(1 - mask[n], kind="stable")
            hs_c[j] = hs[n][perm]
            mperm = mask[n][perm]
            mkb[:, :, j] = np.where(
                mperm.reshape(LT, 128).T == 0, np.float32(-1e4), np.float32(0.0)
            )
        sel9 = np.zeros((9, G, 128), np.float32)
        for c in range(G):
            sel9[2 * (c % 4) + c // 4, c, :] = 1.0
        sel9[8, :, :] = 1.0
        oh = np.zeros((128, 4), np.float32)
        oh[:, 0] = 1.0
        oh[:, 3] = 1.0
        in_maps.append(
            {
                "hs": hs_c,
                "mkb": mkb,
                "sel9": sel9,
                "oh": oh,
                "AT": AT,
                "crow": crow,
            }
        )
    return in_maps, chunk_of, kts


def run_on_device(hidden_states, attention_mask, Wq, bq, Wk, bk, trace=False):
    in_maps, chunk_of, kts = _prepare(
        hidden_states, attention_mask, Wq, bq, Wk, bk
    )
    nc = _get_nc(kts)
    res = run_bass_kernel_spmd(nc, in_maps, core_ids=list(range(NCORES)), trace=trace)
    pooled = np.zeros((N, H), np.float32)
    for core in range(NCORES):
        pooled[chunk_of[core]] = res.results[core]["out"]
    return pooled, res


def kernel(hidden_states, attention_mask, sample_map, Wq, bq, Wk, bk, num_texts):
    pooled, _ = run_on_device(hidden_states, attention_mask, Wq, bq, Wk, bk)
    smap = np.asarray(sample_map).astype(np.int64)
    T = int(num_texts)
    sums = np.zeros((T, H), np.float32)
    np.add.at(sums, smap, pooled)
    counts = np.bincount(smap, minlength=T).astype(np.float32)
    counts = np.clip(counts, 1.0, None)
    return (sums / counts[:, None]).astype(np.float32)


if __name__ == "__main__":
    nc = _build_bass(tuple([3] * NS))
    print("built ok")
